# revision 39
# baseline (speedup 1.0000x reference)
"""Sparse (block-local) attention for B=2, Sq=2048, Sk=4096, D=1024, H=16.

Each query i attends keys {2i, 2i+1}; softmax over 2 scores reduces to a
sigmoid of the score difference.  Algebra used here (per core shard):

  cdiff = c_even - c_odd, cmean = (c_even + c_odd)/2
  ds    = rowdot_head(x @ Wq^T, cdiff @ Wk^T)        # score difference
  t     = tanh(ds * scale / 2)                       # = 2*softmax1 - 1
  out   = cmean @ (Wo Wv)^T + (t ⊙ (cdiff @ Wv^T)) @ (Wo/2)^T

The mean-fold (vs folding v_odd) halves the attention-term magnitude, which
buys the fp8 error budget: Q, Kd, Vd projections run as fp8(e4m3) DoubleRow
matmuls (2 k-tiles per instruction, 2x PE rate).  Weights for fp8 are
host-scaled by 64 to clear the e4m3 subnormal band; the 64*64 score scale
folds into the tanh scale and the Vd scale folds into Wo (Wo/128 on host).
The O projections stay bf16 (fp8 there blows the 2e-2 tolerance).

Distribution: sequence-parallel over (batch, query-block): 8 cores x 512
queries + their 1024 keys.  No collectives; host concatenates.

DMA: inputs are split across three descriptor-generation paths (sync HWDGE,
scalar HWDGE, gpsimd SWDGE) in need-order; output is bf16, chunks alternate
across the two HWDGE rings (host casts to f32).
"""

import sys

for _p in ("/opt/trn_rl_repo",):
    if _p not in sys.path:
        sys.path.append(_p)

import numpy as np
import ml_dtypes

import concourse.bass as bass
import concourse.mybir as mybir
import concourse.tile as tile
from concourse import bacc
from concourse.bass_utils import run_bass_kernel_spmd
from concourse.masks import make_identity
from concourse.tile_rust import add_dep_helper

B, SQ, SK, D, H, HD = 2, 2048, 4096, 1024, 16, 64
N_CORES = 8
QL = B * SQ // N_CORES       # 512 queries per core
KL = 2 * QL                  # 1024 keys per core
QT = QL // 128               # 4 query tiles
NB = 512                     # psum bank width (fp32)
JT = D // NB                 # 2 output-column blocks per projection
DT = D // 128                # 8 feature tiles
SCALE = 1.0 / float(np.sqrt(HD))
WS = 64.0                    # fp8 weight pre-scale (2^6)

FB = mybir.dt.bfloat16
F8 = mybir.dt.float8e4
F32 = mybir.dt.float32
BF = ml_dtypes.bfloat16
E4 = ml_dtypes.float8_e4m3
DR = mybir.MatmulPerfMode.DoubleRow


def _build(kd_tiles: int, with_bo: bool):
    """Build + finalize the per-core Bacc graph (SPMD: same graph on 8 cores)."""
    nc = bacc.Bacc("TRN2", target_bir_lowering=False)

    # All inputs host-arranged partition-major: tensor[p, t, n] =
    # logical[t*128 + p, n]; per-partition data is one contiguous run.
    xq8 = nc.dram_tensor("xq8", [128, kd_tiles, QL], F8, kind="ExternalInput")
    wq8 = nc.dram_tensor("wq8", [128, kd_tiles, D], F8, kind="ExternalInput")
    ck8 = nc.dram_tensor("ck8", [128, kd_tiles, QL + D], F8,
                         kind="ExternalInput")
    wv8 = nc.dram_tensor("wv8", [128, kd_tiles, D], F8, kind="ExternalInput")
    cm = nc.dram_tensor("cm", [128, kd_tiles, QL], FB, kind="ExternalInput")
    wo = nc.dram_tensor("wo", [128, DT, D], FB, kind="ExternalInput")
    wvo = nc.dram_tensor("wvo", [128, kd_tiles, D], FB, kind="ExternalInput")
    bo = None
    if with_bo:
        bo = nc.dram_tensor("bo", [1, D], F32, kind="ExternalInput")
    out = nc.dram_tensor("out", [128, QT, D], FB, kind="ExternalOutput")

    n_dr = kd_tiles // 2
    odd_kd = kd_tiles % 2

    with tile.TileContext(nc) as tc:
        with (
            tc.tile_pool(name="ins", bufs=1) as ins,
            tc.tile_pool(name="acts", bufs=1) as acts,
            tc.tile_pool(name="att", bufs=4) as att,
            tc.tile_pool(name="outs", bufs=4) as outs,
            tc.tile_pool(name="psum", bufs=6, space="PSUM") as psum,
            tc.tile_pool(name="psum_tr", bufs=2, space="PSUM") as psum_tr,
        ):
            # ---- inputs to SBUF: 3 DGE paths, need-order per ring ----------
            xq8_sb = ins.tile([128, kd_tiles, QL], F8)
            wq8_sb = ins.tile([128, kd_tiles, D], F8)
            ck8_sb = ins.tile([128, kd_tiles, QL + D], F8)
            wv8_sb = ins.tile([128, kd_tiles, D], F8)
            cm_sb = ins.tile([128, kd_tiles, QL], FB)
            wo_sb = ins.tile([128, DT, D], FB)
            wvo_sb = ins.tile([128, kd_tiles, D], FB)
            ident = ins.tile([128, 128], FB)

            # HWDGE rings (sync/scalar) outrank SWDGE (gpsimd) in the SDMA
            # round-robin, and each ring drains FIFO — so need-order falls
            # out of ring assignment: early tensors on the two HWDGE rings,
            # late weights on gpsimd (which self-deprioritizes).
            nc.sync.dma_start(out=xq8_sb, in_=xq8[:])
            nc.sync.dma_start(out=ck8_sb, in_=ck8[:])
            d_wq = nc.scalar.dma_start(out=wq8_sb, in_=wq8[:])
            nc.scalar.dma_start(out=wv8_sb, in_=wv8[:])
            # hold the gpsimd stream until wq8 lands: despite its lower SDMA
            # priority it still leaks ~1/3 of the bandwidth, delaying the
            # first Q matmul by ~2.5us
            d_cm = nc.gpsimd.dma_start(out=cm_sb, in_=cm[:])
            d_wvo = nc.gpsimd.dma_start(out=wvo_sb, in_=wvo[:])
            d_wo = nc.gpsimd.dma_start(out=wo_sb, in_=wo[:])
            for d2 in (d_cm, d_wvo, d_wo):
                add_dep_helper(d2.ins, d_wq.ins, sync=True)
            bo_sb = None
            if with_bo:
                bo_sb = ins.tile([128, D], F32)
                nc.gpsimd.dma_start(out=bo_sb,
                                    in_=bo[:].to_broadcast((128, D)))
            make_identity(nc, ident)

            # PE warm-up: 512-wide dummy matmuls bridge the DMA head with NO
            # idle gap before the first real matmul (an idle PE drops back to
            # a low p-state and the first real matmuls run at half clock)
            warm = ins.tile([128, NB], FB)
            nc.vector.memset(warm, 1.0)
            wps = psum.tile([128, NB], F32, tag="mm")
            for _ in range(20):
                nc.tensor.matmul(wps, lhsT=warm[:, 0:128], rhs=warm,
                                 start=True, stop=True)

            # ---- projections (fp8 DoubleRow; psum copies on ACT) -----------
            q_sb = acts.tile([128, QT, D], FB)     # 64*(x Wq^T + bq)
            kd_sb = acts.tile([128, QT, D], FB)    # 64*(cdiff Wk^T)
            v_sb = acts.tile([128, QT, D], FB)     # 64*(cdiff Wv^T)
            av_sb = acts.tile([128, QT, D], FB)    # tanh ⊙ Vd

            def xq_src(kd):
                return xq8_sb, kd

            def wq_src(kd):
                return wq8_sb, kd

            def mm8(dst_tile, qt, jb, lsrc_fn, lq0, rsrc_fn, rq0):
                # *_src_fn(kd) -> (tensor, local kd index)
                ps = psum.tile([128, NB], F32, tag="mm")
                for i in range(n_dr):
                    lt, lo = lsrc_fn(2 * i)
                    rt, ro = rsrc_fn(2 * i)
                    nc.tensor.matmul(
                        ps,
                        lhsT=lt[:, lo:lo + 2,
                                lq0 + qt * 128:lq0 + qt * 128 + 128],
                        rhs=rt[:, ro:ro + 2,
                               rq0 + jb * NB:rq0 + (jb + 1) * NB],
                        perf_mode=DR,
                        start=(i == 0),
                        stop=(i == n_dr - 1 and not odd_kd),
                    )
                if odd_kd:
                    lt, lo = lsrc_fn(kd_tiles - 1)
                    rt, ro = rsrc_fn(kd_tiles - 1)
                    nc.tensor.matmul(
                        ps,
                        lhsT=lt[:, lo, lq0 + qt * 128:lq0 + qt * 128 + 128],
                        rhs=rt[:, ro, rq0 + jb * NB:rq0 + (jb + 1) * NB],
                        start=False,
                        stop=True,
                    )
                nc.scalar.copy(dst_tile[:, qt, jb * NB:(jb + 1) * NB], ps)

            def attention(qt):
                # ds = rowdot(q, kd) per head; t = tanh(ds*scale/2/4096);
                # av = t ⊙ Vd
                qv = q_sb[:, qt, :]
                kdv = kd_sb[:, qt, :]
                pe = att.tile([128, H, HD], FB, tag="prod")
                nc.vector.tensor_mul(pe.rearrange("p h e -> p (h e)"), qv, kdv)
                ds = att.tile([128, H], F32, tag="s")
                nc.vector.reduce_sum(out=ds, in_=pe, axis=mybir.AxisListType.X)
                pt = att.tile([128, H], F32, tag="s")
                nc.scalar.activation(pt, ds, mybir.ActivationFunctionType.Tanh,
                                     scale=SCALE / (2.0 * WS * WS))
                vd = v_sb[:, qt, :].rearrange("p (h e) -> p h e", h=H)
                nc.vector.tensor_mul(
                    av_sb[:, qt, :].rearrange("p (h e) -> p h e", h=H),
                    vd, pt.to_broadcast((128, H, HD)))

            def ck_src(kd):
                return ck8_sb, kd

            def wv_src(kd):
                return wv8_sb, kd

            # Q first: needs only xq8+wq8, the first DMA on each ring
            for jb in range(JT):
                for qt in range(QT):
                    mm8(q_sb, qt, jb, xq_src, 0, wq_src, 0)
            # Kd for all qt, then Vd per qt with attention one qt behind
            for qt in range(QT):
                for jb in range(JT):
                    mm8(kd_sb, qt, jb, ck_src, 0, ck_src, QL)
            for qt in range(QT):
                for jb in range(JT):
                    mm8(v_sb, qt, jb, ck_src, 0, wv_src, 0)
                if qt >= 1:
                    attention(qt - 1)
            attention(QT - 1)

            # ---- transpose av -> avT (copies on DVE), O groups interleaved -
            avT_sb = acts.tile([128, DT, QL], FB)

            def transposes(qt):
                for db in range(DT):
                    tp = psum_tr.tile([128, 128], FB, tag="tr")
                    nc.tensor.transpose(tp, av_sb[:, qt, db * 128:(db + 1) * 128],
                                        ident)
                    nc.vector.tensor_copy(avT_sb[:, db, qt * 128:(qt + 1) * 128],
                                          tp)

            def o_group(qt):
                # out[qt] = cmean @ Wvo^T  (pure half, no attention dep)
                #         + avT^T @ (Wo/128)^T  (attention half)
                # banks interleaved per k-tile so jb0/jb1 finish ~together and
                # their copies+output DMAs drain in parallel (shrinks the
                # final-group tail); per-bank accumulation order is unchanged
                pss = [psum.tile([128, NB], F32, tag="mm", name=f"psg{jb}")
                       for jb in range(JT)]
                for kd in range(kd_tiles):
                    for jb in range(JT):
                        nc.tensor.matmul(
                            pss[jb],
                            lhsT=cm_sb[:, kd, qt * 128:(qt + 1) * 128],
                            rhs=wvo_sb[:, kd, jb * NB:(jb + 1) * NB],
                            start=(kd == 0),
                            stop=False,
                        )
                for kd in range(DT):
                    for jb in range(JT):
                        nc.tensor.matmul(
                            pss[jb],
                            lhsT=avT_sb[:, kd, qt * 128:(qt + 1) * 128],
                            rhs=wo_sb[:, kd, jb * NB:(jb + 1) * NB],
                            start=False,
                            stop=(kd == DT - 1),
                        )
                # jb0 copy on ACT + DMA on sync, jb1 on DVE + scalar: the two
                # halves drain fully in parallel so the last group's
                # write+receipt latency is paid once, not twice
                for jb in range(JT):
                    o_t = outs.tile([128, NB], FB, tag="o")
                    if with_bo:
                        nc.vector.tensor_add(o_t, pss[jb],
                                             bo_sb[:, jb * NB:(jb + 1) * NB])
                    elif jb % 2 == 0:
                        nc.scalar.copy(o_t, pss[jb])
                    else:
                        nc.vector.tensor_copy(o_t, pss[jb])
                    eng = nc.sync if jb % 2 == 0 else nc.scalar
                    eng.dma_start(out=out[:, qt, jb * NB:(jb + 1) * NB],
                                  in_=o_t)

            # PE order keeps PE fed while DVE copies each avT tile group
            transposes(0)
            transposes(1)
            o_group(0)
            transposes(2)
            o_group(1)
            transposes(3)
            o_group(2)
            o_group(3)

    nc.finalize()
    return nc


_GRAPH_CACHE = {}


def _get_graph(kd_tiles: int, with_bo: bool):
    key = (kd_tiles, with_bo)
    if key not in _GRAPH_CACHE:
        _GRAPH_CACHE[key] = _build(kd_tiles, with_bo)
    return _GRAPH_CACHE[key]


def _pmajor(a, kd_tiles):
    """[kd_tiles*128, n] -> [128, kd_tiles, n] partition-major, contiguous."""
    n = a.shape[1]
    return np.ascontiguousarray(
        a.reshape(kd_tiles, 128, n).transpose(1, 0, 2))


def _make_in_maps(x, c, Wq, bq, Wk, bk, Wv, bv, Wo, bo):
    x = np.asarray(x, np.float32)
    c = np.asarray(c, np.float32)
    has_bias = any(np.any(np.asarray(b)) for b in (bq, bk, bv))
    with_bo = bool(np.any(np.asarray(bo)))
    kd_tiles = DT + (1 if has_bias else 0)
    KD = kd_tiles * 128

    def aug_w(W, brow, scale, dt):
        # rows are input features; optional bias row appended
        wT = np.asarray(W, np.float32).T * scale
        if has_bias:
            pad = np.zeros((KD - D, D), np.float32)
            pad[0, :] = np.asarray(brow, np.float32) * scale
            wT = np.concatenate([wT, pad], axis=0)
        return _pmajor(wT.astype(dt), kd_tiles)

    Wo32 = np.asarray(Wo, np.float32)
    Wv32 = np.asarray(Wv, np.float32)
    wq_h = aug_w(Wq, bq, WS, E4)
    wk_h = aug_w(Wk, 0.0 * np.asarray(bk), WS, E4)
    wv_h = aug_w(Wv32, np.zeros(D), WS, E4)
    wvo_h = aug_w(Wo32 @ Wv32, Wo32 @ np.asarray(bv, np.float32), 1.0, BF)
    # att half: out += (tanh ⊙ 64*Vd) @ (0.5*Wo/64)^T ; no bias row (DT tiles)
    wo_h = _pmajor(np.ascontiguousarray(Wo32.T / (2.0 * WS)).astype(BF), DT)

    def aug_act(aT, dt, pad_val):
        if has_bias:
            pad = np.zeros((KD - D, aT.shape[1]), np.float32)
            pad[0, :] = pad_val
            aT = np.concatenate([aT, pad], axis=0)
        return _pmajor(aT.astype(dt), kd_tiles)

    in_maps = []
    for core in range(N_CORES):
        b = core // (N_CORES // B)
        q0 = (core % (N_CORES // B)) * QL
        k0 = 2 * q0
        xs = x[b, q0:q0 + QL]                      # [QL, D]
        cs = c[b, k0:k0 + KL]                      # [KL, D]
        c_mean = (cs[0::2] + cs[1::2]) * 0.5       # [QL, D]
        c_diff = cs[0::2] - cs[1::2]               # [QL, D], fp32 exact
        xT8 = aug_act(np.ascontiguousarray(xs.T), E4, 1.0)
        cmT = aug_act(np.ascontiguousarray(c_mean.T), BF, 1.0)
        cdT8 = aug_act(np.ascontiguousarray(c_diff.T), E4, 0.0)
        m = {
            "xq8": xT8,
            "wq8": wq_h,
            "ck8": np.ascontiguousarray(np.concatenate([cdT8, wk_h], axis=2)),
            "wv8": wv_h,
            "cm": cmT,
            "wo": wo_h,
            "wvo": wvo_h,
        }
        if with_bo:
            m["bo"] = np.asarray(bo, np.float32).reshape(1, D)
        in_maps.append(m)
    return in_maps, kd_tiles, with_bo


def _gather(results):
    out = np.empty((B, SQ, D), np.float32)
    for core in range(N_CORES):
        b = core // (N_CORES // B)
        q0 = (core % (N_CORES // B)) * QL
        # device layout [128, QT, D] -> rows q = qt*128 + p
        arr = np.asarray(results[core]["out"], dtype=np.float32)
        out[b, q0:q0 + QL] = arr.transpose(1, 0, 2).reshape(QL, D)
    return out


def kernel(**inputs) -> np.ndarray:
    in_maps, kd_tiles, with_bo = _make_in_maps(**inputs)
    nc = _get_graph(kd_tiles, with_bo)
    res = run_bass_kernel_spmd(nc, in_maps, core_ids=list(range(N_CORES)))
    return _gather(res.results)


def run_traced(**inputs):
    """Like kernel() but with neuron-profile tracing; returns (out, results)."""
    in_maps, kd_tiles, with_bo = _make_in_maps(**inputs)
    nc = _get_graph(kd_tiles, with_bo)
    res = run_bass_kernel_spmd(nc, in_maps, core_ids=list(range(N_CORES)),
                               trace=True)
    return _gather(res.results), res
